# revision 22
# baseline (speedup 1.0000x reference)
"""DKT (Deep Knowledge Tracing) accumulate-concat model on 8 Trainium2 cores.

Model (per example): one-hot interactions x[t] (2S=1024), query one-hots q,
  emb   = x @ W_emb + b_emb
  count = cumulative count state (c_t = sum(x_t)*c_{t-1} + x_t; x one-hot => cumsum)
  z     = [emb, log1p(count), log1p(delta)]
  h     = LSTM(z)                      (Keras gates i,f,g,o; unit forget bias)
  y     = sum(sigmoid(h @ W_out + b_out) * q, -1)

Sharding: data-parallel over batch. 8 cores x 8 examples. Weights replicated.

Device algorithm per core (B'=8 examples):
  Phase 0: cast weights fp32->fp16, all resident in SBUF (W_lstm^T 40KB/part).
    wdb[2, 2048] = [W_lstm delta row; b_lstm + b_emb @ W1] (bias folded into
    a K=2 matmul against [log1p(delta); ones]).
  Prologue (per example):
    countT[2S, T] = x^T cumsum over t as ONE matmul (lhsT = x, rhs = triu ones).
    zt_all[:, kc, ex, t] = [embT; log1p(countT)] fp16; xt = column-diff of countT.
    ldb[2, ex, t] = [log1p(delta); 1.0]
  Main loop (t = 0..T-1), gates^T layout (partitions = gate cols):
    psum_if/g/o = sum_ko U^T h  (64 MMs of N=8, weight-load bound, FWL)
    pre_if = psum + wz -> ONE fused sigmoid over (i,f) [128,8,8]
    tanh g; c = sig_f*c + sig_i*tanh_g; after o-MMs: sigmoid o; h = sig_o*tanh(c)
    FILLER: the big WzT = W_lstm^T z matmul for time-chunk c+1 is interleaved
    into the PE-idle tail of each step (paced 3 ops/step, N=512 MMs, psum
    accumulated across steps). Keeps the PE busy (p-state ramp) and hides all
    of the Wz cost. wz chunk buffers [128,16,8,CH] fp16, double-buffered.
  Phase 3 (fused, every 16 steps): s = h @ W_out + b_out, y = sum(sig(s)*q).

Output DRAM tensor is [T, 8] (t-major) for contiguous stores; host transposes.
"""

import os
import sys

sys.path.insert(0, "/opt/trn_rl_repo")

import numpy as np

import concourse.bass as bass
import concourse.tile as tile
from concourse import bacc, mybir
from concourse.bass_utils import run_bass_kernel_spmd

F32 = mybir.dt.float32
F16 = mybir.dt.float16
F8 = mybir.dt.float8e4
AF = mybir.ActivationFunctionType
ALU = mybir.AluOpType

N_CORES = 8
B_FULL, T_FULL, S = 64, 512, 512
S2 = 2 * S          # 1024 one-hot dim
DE = 256            # emb dim
H = 512             # lstm hidden
G4 = 4 * H          # 2048 gate cols
BP = 8              # examples per core
CH = 64             # Wz time-chunk (filler granularity)


def _build(T=T_FULL):
    KT = T // 128           # K-tiles over time for count matmul
    nc = bacc.Bacc("TRN2", target_bir_lowering=False, debug=False)

    x_h = nc.dram_tensor("x", [BP, T, S2], F32, kind="ExternalInput")
    d_h = nc.dram_tensor("delta", [BP, T], F32, kind="ExternalInput")
    q_h = nc.dram_tensor("q", [BP, T, S], F32, kind="ExternalInput")
    we_h = nc.dram_tensor("W_emb", [S2, DE], F32, kind="ExternalInput")
    be_h = nc.dram_tensor("b_emb", [DE], F32, kind="ExternalInput")
    wl_h = nc.dram_tensor("W_lstm", [S2 + DE + 1, G4], F32, kind="ExternalInput")
    ul_h = nc.dram_tensor("U_lstm", [H, G4], F32, kind="ExternalInput")
    bl_h = nc.dram_tensor("b_lstm", [G4], F32, kind="ExternalInput")
    wo_h = nc.dram_tensor("W_out", [H, S], F32, kind="ExternalInput")
    bo_h = nc.dram_tensor("b_out", [S], F32, kind="ExternalInput")
    y_h = nc.dram_tensor("y", [T, BP], F32, kind="ExternalOutput")

    tri_h = nc.inline_tensor(np.triu(np.ones((T, T), np.float16)), name="triu")

    x, d, q = x_h.ap(), d_h.ap(), q_h.ap()
    we, be, wl, ul, bl = we_h.ap(), be_h.ap(), wl_h.ap(), ul_h.ap(), bl_h.ap()
    wo, bo, y, tri = wo_h.ap(), bo_h.ap(), y_h.ap(), tri_h.ap()

    with tile.TileContext(nc) as tc:
        _kernel_body(nc, tc, T, KT, x, d, q, we, be, wl, ul, bl, wo, bo, y, tri)
    nc.compile()
    return nc


def _kernel_body(nc, tc, T, KT, x, d, q, we, be, wl, ul, bl, wo, bo, y, tri):
    from contextlib import ExitStack

    NCH = T // CH
    ctx = ExitStack()
    with ctx:
        # ---------- persistent pools ----------
        per = ctx.enter_context(tc.tile_pool(name="persist", bufs=1))
        wz_pool = ctx.enter_context(tc.tile_pool(name="wz", bufs=1))

        # W_lstm^T resident fp16: [128 k-rows, kc=10, 2048 gate cols] 40KB/part
        wl_sb = per.tile([128, 10, G4], F16)
        u_sb = per.tile([128, 4, G4], F16)                # 16KB/part
        wo_sb = per.tile([128, 4, S], F16)                # 4KB/part
        # delta+bias rows: [2, ex, T] fp16 (row0 log1p(delta), row1 ones)
        ldb = per.tile([2, BP, T], F16)
        # K=2 lhsT rows: row0 = W_lstm delta row, row1 = b_lstm + b_emb @ W1
        wdb = per.tile([2, G4], F16)
        bembT = per.tile([128, 2], F32)
        bembT16 = per.tile([128, 2], F16)
        bout16 = per.tile([1, S], F16)
        ones1 = per.tile([1, 128], F16)
        zero1 = per.tile([1, 128], F16)
        nc.vector.memset(ones1, 1.0)
        nc.vector.memset(zero1, 0.0)
        nc.vector.memset(ldb, 1.0)      # row 0 overwritten per-example below

        # ---------- phase 0: load + cast weights ----------
        # single shared staging tag keeps the pool at 8KB/part
        with tc.tile_pool(name="ph0", bufs=2) as p0, \
             tc.tile_pool(name="ph0ps", bufs=2, space="PSUM") as p0ps:
            def stage(dst, src_ap, rows=128):
                t32 = p0.tile([128, 1024], F32, tag="stg")
                nc.sync.dma_start(out=t32[0:rows, 0:src_ap.shape[-1]],
                                  in_=src_ap)
                nc.scalar.activation(dst, t32[0:rows, 0:src_ap.shape[-1]],
                                     AF.Copy)
            for ko in range(4):
                for hh in range(2):
                    stage(u_sb[:, ko, 1024 * hh:1024 * (hh + 1)],
                          ul[128 * ko:128 * (ko + 1),
                             1024 * hh:1024 * (hh + 1)])
            for ko in range(4):
                stage(wo_sb[:, ko, :], wo[128 * ko:128 * (ko + 1), :])
            # W_lstm rows 0..1279 -> resident SBUF fp16
            for kc in range(10):
                for hh in range(2):
                    stage(wl_sb[:, kc, 1024 * hh:1024 * (hh + 1)],
                          wl[128 * kc:128 * (kc + 1),
                             1024 * hh:1024 * (hh + 1)])
            # delta row of W_lstm -> wdb row 0
            for hh in range(2):
                stage(wdb[0:1, 1024 * hh:1024 * (hh + 1)],
                      wl[1280:1281, 1024 * hh:1024 * (hh + 1)], rows=1)
            # biases
            t32 = p0.tile([128, 1024], F32, tag="stg")
            nc.sync.dma_start(
                out=t32[:, 0:2],
                in_=bass.AP(tensor=be.tensor, offset=be.offset,
                            ap=[[1, 128], [128, 2]]),
            )
            nc.vector.tensor_copy(bembT, t32[:, 0:2])
            nc.vector.tensor_copy(bembT16, t32[:, 0:2])
            stage(bout16, bo[None, :], rows=1)
            # wdb row 1 = b_lstm + b_emb @ W1, built on partition 0 then
            # DMA'd into partition 1 (engine ops can't start at partition 1)
            for hh in range(2):
                blh = p0.tile([128, 1024], F32, tag="stg")
                nc.sync.dma_start(out=blh[0:1, :],
                                  in_=bl[None, 1024 * hh:1024 * (hh + 1)])
                b16h = p0.tile([1, 1024], F16, tag="b16")
                for s2 in range(2):
                    seg = 2 * hh + s2
                    seg_ps = p0ps.tile([1, 512], F32, tag="segps")
                    for kc in range(2):
                        nc.tensor.matmul(
                            seg_ps, bembT16[:, kc:kc + 1],
                            wl_sb[:, kc, 512 * seg:512 * (seg + 1)],
                            start=(kc == 0), stop=(kc == 1))
                    nc.vector.tensor_add(
                        b16h[0:1, 512 * s2:512 * (s2 + 1)], seg_ps,
                        blh[0:1, 512 * s2:512 * (s2 + 1)])
                nc.sync.dma_start(out=wdb[1:2, 1024 * hh:1024 * (hh + 1)],
                                  in_=b16h)

        # z^T for all examples: [128, kc=10, ex=8, T] fp16  80KB/part
        zt_all = per.tile([128, 10, BP, T], F16)

        # ---------- prologue: count/emb/log terms for all examples ----------
        with tc.tile_pool(name="pro", bufs=1) as p1, \
             tc.tile_pool(name="prox", bufs=2) as p1x, \
             tc.tile_pool(name="procnt", bufs=1, space="PSUM") as pps, \
             tc.tile_pool(name="promm", bufs=1, space="PSUM") as pps2:
            wemb = p1.tile([128, 8, DE], F16, tag="wemb")
            tri_sb = p1.tile([128, KT, T], F16, tag="trisb")
            for mc in range(8):
                t32 = p1x.tile([128, DE], F32, tag="we32")
                nc.sync.dma_start(out=t32, in_=we[128 * mc:128 * (mc + 1), :])
                nc.scalar.activation(wemb[:, mc, :], t32, AF.Copy)
            for kt in range(KT):
                nc.sync.dma_start(out=tri_sb[:, kt, :],
                                  in_=tri[128 * kt:128 * (kt + 1), :])
            for ex in range(BP):
                ld32 = p1.tile([1, T], F32, tag="ld32")
                nc.sync.dma_start(out=ld32, in_=d[ex:ex + 1, :])
                nc.scalar.activation(ldb[0:1, ex, :], ld32, AF.Ln, bias=1.0)
                # -- count matmul + log1p + diff + emb accumulate --
                e_ps = [pps2.tile([128, T], F32, tag=f"emb{m2}",
                                  name=f"emb{m2}") for m2 in range(2)]
                for mch in range(2):
                    cnt_ps = [pps.tile([128, T], F32, tag=f"cnt{i}",
                                       name=f"cnt{i}")
                              for i in range(4)]
                    for kt in range(KT):
                        # x columns [512*mch, 512*(mch+1)) of this kt tile
                        x32 = p1x.tile([128, S], F32, tag="x32")
                        nc.sync.dma_start(
                            out=x32,
                            in_=x[ex, 128 * kt:128 * (kt + 1),
                                  S * mch:S * (mch + 1)])
                        x16 = p1x.tile([128, S], F16, tag="x16")
                        nc.scalar.activation(x16, x32, AF.Copy)
                        for i in range(4):
                            nc.tensor.matmul(
                                cnt_ps[i], x16[:, 128 * i:128 * (i + 1)],
                                tri_sb[:, kt, :],
                                start=(kt == 0), stop=(kt == KT - 1))
                    for i in range(4):
                        mc = 4 * mch + i
                        nc.scalar.activation(zt_all[:, 2 + mc, ex, :],
                                             cnt_ps[i], AF.Ln, bias=1.0)
                        cnt_sb = p1x.tile([128, T], F16, tag="cntsb")
                        nc.vector.tensor_copy(cnt_sb, cnt_ps[i])
                        xt = p1x.tile([128, T], F16, tag="xt")
                        nc.vector.tensor_copy(xt[:, 0:1], cnt_sb[:, 0:1])
                        nc.vector.tensor_sub(xt[:, 1:T], cnt_sb[:, 1:T],
                                             cnt_sb[:, 0:T - 1])
                        for m2 in range(2):
                            nc.tensor.matmul(
                                e_ps[m2],
                                wemb[:, mc, 128 * m2:128 * (m2 + 1)],
                                xt,
                                start=(mc == 0), stop=(mc == 7))
                for m2 in range(2):
                    nc.vector.tensor_scalar_add(zt_all[:, m2, ex, :],
                                                e_ps[m2],
                                                bembT[:, m2:m2 + 1])

        # wz chunk double buffer: [128, m=16, ex=8, CH] fp16, 16KB/part each
        wz = [wz_pool.tile([128, 16, BP, CH], F16, name=f"wzbuf{i}")
              for i in range(2)]

        # ---------- Wz chunk program (generator, consumed as filler) -------
        wz_ps_pool = ctx.enter_context(
            tc.tile_pool(name="wzps", bufs=2, space="PSUM"))

        def wz_chunk_ops(c):
            """Yield thunks; each emits one Tensor-engine MM (or the
            finalize copy) for wz chunk c into buffer wz[c % 2]."""
            t0 = c * CH
            buf = wz[c % 2]
            for m in range(16):
                b_ps = wz_ps_pool.tile([128, BP, CH], F32, tag="wzps")
                for kc in range(10):
                    def mm(kc=kc, m=m, b_ps=b_ps):
                        nc.tensor.matmul(
                            b_ps, wl_sb[:, kc, 128 * m:128 * (m + 1)],
                            zt_all[:, kc, :, t0:t0 + CH],
                            start=(kc == 0), stop=False)
                    yield mm

                def mm_last(m=m, b_ps=b_ps, t0=t0):
                    nc.tensor.matmul(b_ps, wdb[:, 128 * m:128 * (m + 1)],
                                     ldb[:, :, t0:t0 + CH],
                                     start=False, stop=True)
                yield mm_last

                def fin(m=m, b_ps=b_ps, buf=buf):
                    if m % 2 == 0:
                        nc.vector.tensor_copy(buf[:, m, :, :], b_ps)
                    else:
                        nc.scalar.activation(buf[:, m, :, :], b_ps, AF.Copy)
                yield fin

        # ---------- main loop: recurrence + filler ----------
        with tc.tile_pool(name="rec", bufs=1) as rp, \
             tc.tile_pool(name="recd", bufs=2) as rd, \
             tc.tile_pool(name="act4", bufs=2) as ap4, \
             tc.tile_pool(name="gps", bufs=2, space="PSUM") as gps, \
             tc.tile_pool(name="sps", bufs=2, space="PSUM") as sps:
            hring = rp.tile([128, 4, 33, BP], F16)
            c0 = rp.tile([128, 4, BP], F32)
            nc.vector.memset(hring[:, :, 0, :], 0.0)
            nc.vector.memset(c0, 0.0)
            cprev = c0

            # chunk 0 wz: emit inline before the loop (prologue tail)
            for op in wz_chunk_ops(0):
                op()
            filler = wz_chunk_ops(1) if NCH > 1 else iter(())
            filler_chunk = 1
            emitted = 0

            for t in range(T):
                tc_idx = t % CH
                cbuf = wz[(t // CH) % 2]
                sl_prev = 1 + ((t - 1) % 32) if t > 0 else 0
                sl = 1 + (t % 32)

                # all four gate psums in ONE tile; wz pre-loaded into PSUM
                # (zero dummy MM sets has_written, one DVE cast drops wz in,
                # gate MMs accumulate with start=False) so activations read
                # PSUM directly - no pre-adds on the critical chain.
                pall = gps.tile([128, 16, BP], F32, tag="pall", name="pall")
                nc.tensor.matmul(pall, zero1, ones1, start=True, stop=False)
                nc.vector.tensor_copy(pall, cbuf[:, :, :, tc_idx])

                def gate_mms(jo0):
                    for jo in range(4):
                        m = jo0 + jo
                        for ko in range(4):
                            nc.tensor.matmul(
                                pall[:, m, :],
                                u_sb[:, ko, 128 * m:128 * (m + 1)],
                                hring[:, ko, sl_prev, :],
                                start=False, stop=(ko == 3))

                s_if = ap4.tile([128, 8, BP], F32, tag="sif")
                gate_mms(0)                  # i
                nc.scalar.activation(s_if[:, 0:4, :], pall[:, 0:4, :],
                                     AF.Sigmoid)
                gate_mms(4)                  # f
                nc.scalar.activation(s_if[:, 4:8, :], pall[:, 4:8, :],
                                     AF.Sigmoid)
                cf = ap4.tile([128, 4, BP], F32, tag="cf")
                nc.vector.tensor_mul(cf, s_if[:, 4:8, :], cprev)

                gate_mms(8)                  # g
                tg = ap4.tile([128, 4, BP], F32, tag="tg")
                nc.scalar.activation(tg, pall[:, 8:12, :], AF.Tanh)
                ig = ap4.tile([128, 4, BP], F32, tag="ig")
                nc.vector.tensor_mul(ig, s_if[:, 0:4, :], tg)
                cnew = ap4.tile([128, 4, BP], F32, tag="c")
                nc.vector.tensor_add(cnew, cf, ig)
                th = ap4.tile([128, 4, BP], F32, tag="th")
                nc.scalar.activation(th, cnew, AF.Tanh)

                gate_mms(12)                 # o
                so = ap4.tile([128, 4, BP], F32, tag="so")
                nc.scalar.activation(so, pall[:, 12:16, :], AF.Sigmoid)
                nc.vector.tensor_mul(hring[:, :, sl, :], so, th)
                cprev = cnew

                # filler: paced Wz ops for the upcoming chunk (3/step
                # exact), issued after the tail so tail ops don't wait
                target = 3 * (tc_idx + 1)
                while emitted < target:
                    op = next(filler, None)
                    if op is None:
                        break
                    op()
                    emitted += 1
                if tc_idx == CH - 1:
                    for op in filler:       # drain (should be empty)
                        op()
                    filler_chunk += 1
                    if filler_chunk < NCH:
                        filler = wz_chunk_ops(filler_chunk)
                    else:
                        filler = iter(())
                    emitted = 0

                if t % 16 == 15:
                    t0 = t - 15
                    sl0 = 1 + (t0 % 32)
                    s_ps = sps.tile([128, S], F32, tag="sps")
                    for ko in range(4):
                        nc.tensor.matmul(
                            s_ps, hring[:, ko, sl0:sl0 + 16, :],
                            wo_sb[:, ko, :],
                            start=(ko == 0), stop=False)
                    nc.tensor.matmul(s_ps, ones1, bout16,
                                     start=False, stop=True)
                    sig = rd.tile([128, S], F32, tag="sig")
                    nc.scalar.activation(sig, s_ps, AF.Sigmoid)
                    q_t = rd.tile([128, S], F32, tag="qt")
                    nc.sync.dma_start(
                        out=q_t,
                        in_=bass.AP(tensor=q.tensor,
                                    offset=q.offset + t0 * S,
                                    ap=[[S, 16], [T * S, BP], [1, S]]),
                    )
                    ycol = rd.tile([128, 1], F32, tag="ycol")
                    nc.vector.tensor_mul(sig, sig, q_t)
                    nc.vector.tensor_reduce(ycol, sig, mybir.AxisListType.X,
                                            ALU.add)
                    nc.sync.dma_start(out=y[t0:t0 + 16, :], in_=ycol)


_CACHE = {}


def _get_nc(T=T_FULL):
    if T not in _CACHE:
        _CACHE[T] = _build(T)
    return _CACHE[T]


def kernel(x, delta, q, W_emb, b_emb, W_lstm, U_lstm, b_lstm, W_out, b_out):
    T = x.shape[1]
    nc = _get_nc(T)
    shared = dict(
        W_emb=np.ascontiguousarray(W_emb, np.float32),
        b_emb=np.ascontiguousarray(b_emb, np.float32),
        W_lstm=np.ascontiguousarray(W_lstm, np.float32),
        U_lstm=np.ascontiguousarray(U_lstm, np.float32),
        b_lstm=np.ascontiguousarray(b_lstm, np.float32),
        W_out=np.ascontiguousarray(W_out, np.float32),
        b_out=np.ascontiguousarray(b_out, np.float32),
    )
    in_maps = []
    for c in range(N_CORES):
        sl = slice(BP * c, BP * (c + 1))
        in_maps.append(dict(
            x=np.ascontiguousarray(x[sl], np.float32),
            delta=np.ascontiguousarray(np.asarray(delta)[sl, :, 0], np.float32),
            q=np.ascontiguousarray(q[sl], np.float32),
            **shared,
        ))
    res = run_bass_kernel_spmd(nc, in_maps, core_ids=list(range(N_CORES)))
    out = np.empty((x.shape[0], T, 1), np.float32)
    for c in range(N_CORES):
        out[BP * c:BP * (c + 1), :, 0] = res.results[c]["y"].T
    return out


# revision 25
# speedup vs baseline: 1.2935x; 1.2935x over previous
"""DKT (Deep Knowledge Tracing) accumulate-concat model on 8 Trainium2 cores.

Model (per example): one-hot interactions x[t] (2S=1024), query one-hots q,
  emb   = x @ W_emb + b_emb
  count = cumulative count state (c_t = sum(x_t)*c_{t-1} + x_t; x one-hot => cumsum)
  z     = [emb, log1p(count), log1p(delta)]
  h     = LSTM(z)                      (Keras gates i,f,g,o; unit forget bias)
  y     = sum(sigmoid(h @ W_out + b_out) * q, -1)

Sharding: data-parallel over batch. 8 cores x 8 examples. Weights replicated.

Device algorithm per core (B'=8 examples):
  Phase 0: cast weights fp32->fp16, all resident in SBUF (W_lstm^T 40KB/part).
    wdb[2, 2048] = [W_lstm delta row; b_lstm + b_emb @ W1] (bias folded into
    a K=2 matmul against [log1p(delta); ones]).
  Prologue (per example):
    countT[2S, T] = x^T cumsum over t as ONE matmul (lhsT = x, rhs = triu ones).
    zt_all[:, kc, ex, t] = [embT; log1p(countT)] fp16; xt = column-diff of countT.
    ldb[2, ex, t] = [log1p(delta); 1.0]
  Main loop (t = 0..T-1), gates^T layout (partitions = gate cols):
    psum_if/g/o = sum_ko U^T h  (64 MMs of N=8, weight-load bound, FWL)
    pre_if = psum + wz -> ONE fused sigmoid over (i,f) [128,8,8]
    tanh g; c = sig_f*c + sig_i*tanh_g; after o-MMs: sigmoid o; h = sig_o*tanh(c)
    FILLER: the big WzT = W_lstm^T z matmul for time-chunk c+1 is interleaved
    into the PE-idle tail of each step (paced 3 ops/step, N=512 MMs, psum
    accumulated across steps). Keeps the PE busy (p-state ramp) and hides all
    of the Wz cost. wz chunk buffers [128,16,8,CH] fp16, double-buffered.
  Phase 3 (fused, every 16 steps): s = h @ W_out + b_out, y = sum(sig(s)*q).

Output DRAM tensor is [T, 8] (t-major) for contiguous stores; host transposes.
"""

import os
import sys

sys.path.insert(0, "/opt/trn_rl_repo")

import numpy as np

import concourse.bass as bass
import concourse.tile as tile
from concourse import bacc, mybir
from concourse.bass_utils import run_bass_kernel_spmd

F32 = mybir.dt.float32
F16 = mybir.dt.float16
F8 = mybir.dt.float8e4
AF = mybir.ActivationFunctionType
ALU = mybir.AluOpType

N_CORES = 8
B_FULL, T_FULL, S = 64, 512, 512
S2 = 2 * S          # 1024 one-hot dim
DE = 256            # emb dim
H = 512             # lstm hidden
G4 = 4 * H          # 2048 gate cols
BP = 8              # examples per core
CH = 64             # Wz time-chunk (filler granularity)


def _build(T=T_FULL):
    KT = T // 128           # K-tiles over time for count matmul
    nc = bacc.Bacc("TRN2", target_bir_lowering=False, debug=False)

    x_h = nc.dram_tensor("x", [BP, T, S2], F32, kind="ExternalInput")
    d_h = nc.dram_tensor("delta", [BP, T], F32, kind="ExternalInput")
    q_h = nc.dram_tensor("q", [BP, T, S], F32, kind="ExternalInput")
    we_h = nc.dram_tensor("W_emb", [S2, DE], F32, kind="ExternalInput")
    be_h = nc.dram_tensor("b_emb", [DE], F32, kind="ExternalInput")
    wl_h = nc.dram_tensor("W_lstm", [S2 + DE + 1, G4], F32, kind="ExternalInput")
    ul_h = nc.dram_tensor("U_lstm", [H, G4], F32, kind="ExternalInput")
    bl_h = nc.dram_tensor("b_lstm", [G4], F32, kind="ExternalInput")
    wo_h = nc.dram_tensor("W_out", [H, S], F32, kind="ExternalInput")
    bo_h = nc.dram_tensor("b_out", [S], F32, kind="ExternalInput")
    y_h = nc.dram_tensor("y", [T, BP], F32, kind="ExternalOutput")

    tri_h = nc.inline_tensor(np.triu(np.ones((T, T), np.float16)), name="triu")

    x, d, q = x_h.ap(), d_h.ap(), q_h.ap()
    we, be, wl, ul, bl = we_h.ap(), be_h.ap(), wl_h.ap(), ul_h.ap(), bl_h.ap()
    wo, bo, y, tri = wo_h.ap(), bo_h.ap(), y_h.ap(), tri_h.ap()

    with tile.TileContext(nc) as tc:
        _kernel_body(nc, tc, T, KT, x, d, q, we, be, wl, ul, bl, wo, bo, y, tri)
    nc.compile()
    return nc


def _kernel_body(nc, tc, T, KT, x, d, q, we, be, wl, ul, bl, wo, bo, y, tri):
    from contextlib import ExitStack

    NCH = T // CH
    ctx = ExitStack()
    with ctx:
        # ---------- persistent pools ----------
        per = ctx.enter_context(tc.tile_pool(name="persist", bufs=1))
        wz_pool = ctx.enter_context(tc.tile_pool(name="wz", bufs=1))

        # W_lstm^T resident fp16: [128 k-rows, kc=10, 2048 gate cols] 40KB/part
        wl_sb = per.tile([128, 10, G4], F16)
        u_sb = per.tile([128, 4, G4], F16)                # 16KB/part
        wo_sb = per.tile([128, 4, S], F16)                # 4KB/part
        # delta+bias rows: [2, ex, T] fp16 (row0 log1p(delta), row1 ones)
        ldb = per.tile([2, BP, T], F16)
        # K=2 lhsT rows: row0 = W_lstm delta row, row1 = b_lstm + b_emb @ W1
        wdb = per.tile([2, G4], F16)
        bembT = per.tile([128, 2], F32)
        bembT16 = per.tile([128, 2], F16)
        bout16 = per.tile([1, S], F16)
        ones1 = per.tile([1, 128], F16)
        zero1 = per.tile([1, 128], F16)
        nc.vector.memset(ones1, 1.0)
        nc.vector.memset(zero1, 0.0)
        nc.vector.memset(ldb, 1.0)      # row 0 overwritten per-example below

        # ---------- phase 0: load + cast weights ----------
        # single shared staging tag keeps the pool at 8KB/part
        with tc.tile_pool(name="ph0", bufs=2) as p0, \
             tc.tile_pool(name="ph0ps", bufs=2, space="PSUM") as p0ps:
            def stage(dst, src_ap, rows=128):
                t32 = p0.tile([128, 1024], F32, tag="stg")
                nc.sync.dma_start(out=t32[0:rows, 0:src_ap.shape[-1]],
                                  in_=src_ap)
                nc.scalar.activation(dst, t32[0:rows, 0:src_ap.shape[-1]],
                                     AF.Copy)
            for ko in range(4):
                for hh in range(2):
                    stage(u_sb[:, ko, 1024 * hh:1024 * (hh + 1)],
                          ul[128 * ko:128 * (ko + 1),
                             1024 * hh:1024 * (hh + 1)])
            for ko in range(4):
                stage(wo_sb[:, ko, :], wo[128 * ko:128 * (ko + 1), :])
            # W_lstm rows 0..1279 -> resident SBUF fp16
            for kc in range(10):
                for hh in range(2):
                    stage(wl_sb[:, kc, 1024 * hh:1024 * (hh + 1)],
                          wl[128 * kc:128 * (kc + 1),
                             1024 * hh:1024 * (hh + 1)])
            # delta row of W_lstm -> wdb row 0
            for hh in range(2):
                stage(wdb[0:1, 1024 * hh:1024 * (hh + 1)],
                      wl[1280:1281, 1024 * hh:1024 * (hh + 1)], rows=1)
            # biases
            t32 = p0.tile([128, 1024], F32, tag="stg")
            nc.sync.dma_start(
                out=t32[:, 0:2],
                in_=bass.AP(tensor=be.tensor, offset=be.offset,
                            ap=[[1, 128], [128, 2]]),
            )
            nc.vector.tensor_copy(bembT, t32[:, 0:2])
            nc.vector.tensor_copy(bembT16, t32[:, 0:2])
            stage(bout16, bo[None, :], rows=1)
            # wdb row 1 = b_lstm + b_emb @ W1, built on partition 0 then
            # DMA'd into partition 1 (engine ops can't start at partition 1)
            for hh in range(2):
                blh = p0.tile([128, 1024], F32, tag="stg")
                nc.sync.dma_start(out=blh[0:1, :],
                                  in_=bl[None, 1024 * hh:1024 * (hh + 1)])
                b16h = p0.tile([1, 1024], F16, tag="b16")
                for s2 in range(2):
                    seg = 2 * hh + s2
                    seg_ps = p0ps.tile([1, 512], F32, tag="segps")
                    for kc in range(2):
                        nc.tensor.matmul(
                            seg_ps, bembT16[:, kc:kc + 1],
                            wl_sb[:, kc, 512 * seg:512 * (seg + 1)],
                            start=(kc == 0), stop=(kc == 1))
                    nc.vector.tensor_add(
                        b16h[0:1, 512 * s2:512 * (s2 + 1)], seg_ps,
                        blh[0:1, 512 * s2:512 * (s2 + 1)])
                nc.sync.dma_start(out=wdb[1:2, 1024 * hh:1024 * (hh + 1)],
                                  in_=b16h)

        # z^T for all examples: [128, kc=10, ex=8, T] fp16  80KB/part
        zt_all = per.tile([128, 10, BP, T], F16)

        # ---------- prologue: count/emb/log terms for all examples ----------
        with tc.tile_pool(name="pro", bufs=1) as p1, \
             tc.tile_pool(name="prox", bufs=2) as p1x, \
             tc.tile_pool(name="procnt", bufs=1, space="PSUM") as pps, \
             tc.tile_pool(name="promm", bufs=1, space="PSUM") as pps2:
            wemb = p1.tile([128, 8, DE], F16, tag="wemb")
            tri_sb = p1.tile([128, KT, T], F16, tag="trisb")
            for mc in range(8):
                t32 = p1x.tile([128, DE], F32, tag="we32")
                nc.sync.dma_start(out=t32, in_=we[128 * mc:128 * (mc + 1), :])
                nc.scalar.activation(wemb[:, mc, :], t32, AF.Copy)
            for kt in range(KT):
                nc.sync.dma_start(out=tri_sb[:, kt, :],
                                  in_=tri[128 * kt:128 * (kt + 1), :])
            for ex in range(BP):
                ld32 = p1.tile([1, T], F32, tag="ld32")
                nc.sync.dma_start(out=ld32, in_=d[ex:ex + 1, :])
                nc.scalar.activation(ldb[0:1, ex, :], ld32, AF.Ln, bias=1.0)
                # -- count matmul + log1p + diff + emb accumulate --
                e_ps = [pps2.tile([128, T], F32, tag=f"emb{m2}",
                                  name=f"emb{m2}") for m2 in range(2)]
                for mch in range(2):
                    cnt_ps = [pps.tile([128, T], F32, tag=f"cnt{i}",
                                       name=f"cnt{i}")
                              for i in range(4)]
                    for kt in range(KT):
                        # x columns [512*mch, 512*(mch+1)) of this kt tile
                        x32 = p1x.tile([128, S], F32, tag="x32")
                        nc.sync.dma_start(
                            out=x32,
                            in_=x[ex, 128 * kt:128 * (kt + 1),
                                  S * mch:S * (mch + 1)])
                        x16 = p1x.tile([128, S], F16, tag="x16")
                        nc.scalar.activation(x16, x32, AF.Copy)
                        for i in range(4):
                            nc.tensor.matmul(
                                cnt_ps[i], x16[:, 128 * i:128 * (i + 1)],
                                tri_sb[:, kt, :],
                                start=(kt == 0), stop=(kt == KT - 1))
                    for i in range(4):
                        mc = 4 * mch + i
                        nc.scalar.activation(zt_all[:, 2 + mc, ex, :],
                                             cnt_ps[i], AF.Ln, bias=1.0)
                        cnt_sb = p1x.tile([128, T], F16, tag="cntsb")
                        nc.vector.tensor_copy(cnt_sb, cnt_ps[i])
                        xt = p1x.tile([128, T], F16, tag="xt")
                        nc.vector.tensor_copy(xt[:, 0:1], cnt_sb[:, 0:1])
                        nc.vector.tensor_sub(xt[:, 1:T], cnt_sb[:, 1:T],
                                             cnt_sb[:, 0:T - 1])
                        for m2 in range(2):
                            nc.tensor.matmul(
                                e_ps[m2],
                                wemb[:, mc, 128 * m2:128 * (m2 + 1)],
                                xt,
                                start=(mc == 0), stop=(mc == 7))
                for m2 in range(2):
                    nc.vector.tensor_scalar_add(zt_all[:, m2, ex, :],
                                                e_ps[m2],
                                                bembT[:, m2:m2 + 1])

        # wz chunk double buffer: [128, m=16, ex=8, CH] fp16, 16KB/part each
        wz = [wz_pool.tile([128, 16, BP, CH], F16, name=f"wzbuf{i}")
              for i in range(2)]

        # ---------- Wz chunk program (generator, consumed as filler) -------
        wz_ps_pool = ctx.enter_context(
            tc.tile_pool(name="wzps", bufs=2, space="PSUM"))

        def wz_chunk_ops(c):
            """Yield thunks; each emits one Tensor-engine MM (or the
            finalize copy) for wz chunk c into buffer wz[c % 2]."""
            t0 = c * CH
            buf = wz[c % 2]
            for m in range(16):
                b_ps = wz_ps_pool.tile([128, BP, CH], F32, tag="wzps")
                for kc in range(10):
                    def mm(kc=kc, m=m, b_ps=b_ps):
                        nc.tensor.matmul(
                            b_ps, wl_sb[:, kc, 128 * m:128 * (m + 1)],
                            zt_all[:, kc, :, t0:t0 + CH],
                            start=(kc == 0), stop=False)
                    yield mm

                def mm_last(m=m, b_ps=b_ps, t0=t0):
                    nc.tensor.matmul(b_ps, wdb[:, 128 * m:128 * (m + 1)],
                                     ldb[:, :, t0:t0 + CH],
                                     start=False, stop=True)
                yield mm_last

                def fin(m=m, b_ps=b_ps, buf=buf):
                    if m % 2 == 0:
                        nc.vector.tensor_copy(buf[:, m, :, :], b_ps)
                    else:
                        nc.scalar.activation(buf[:, m, :, :], b_ps, AF.Copy)
                yield fin

        # ---------- main loop: recurrence + filler ----------
        with tc.tile_pool(name="rec", bufs=1) as rp, \
             tc.tile_pool(name="recd", bufs=2) as rd, \
             tc.tile_pool(name="act4", bufs=2) as ap4, \
             tc.tile_pool(name="gps", bufs=1, space="PSUM") as gps, \
             tc.tile_pool(name="sps", bufs=2, space="PSUM") as sps:
            hring = rp.tile([128, 4, 33, BP], F16)
            c0 = rp.tile([128, 4, BP], F32)
            nc.vector.memset(hring[:, :, 0, :], 0.0)
            nc.vector.memset(c0, 0.0)
            cprev = c0

            # chunk 0 wz: emit inline before the loop (prologue tail)
            for op in wz_chunk_ops(0):
                op()
            filler = wz_chunk_ops(1) if NCH > 1 else iter(())
            filler_chunk = 1
            emitted = 0

            for t in range(T):
                tc_idx = t % CH
                cbuf = wz[(t // CH) % 2]
                sl_prev = 1 + ((t - 1) % 32) if t > 0 else 0
                sl = 1 + (t % 32)

                # per-gate psum tiles; issue order (g, i, f, o): the
                # c-chain (tanh g -> c -> tanh c) overlaps the remaining
                # gate matmuls; only the short o-path trails the last MM.
                for g in (2, 0, 1, 3):
                    g_ps = gps.tile([128, 4, BP], F32, tag=f"g{g}",
                                    name=f"g{g}")
                    for jo in range(4):
                        m = 4 * g + jo
                        for ko in range(4):
                            nc.tensor.matmul(
                                g_ps[:, jo, :],
                                u_sb[:, ko, 128 * m:128 * (m + 1)],
                                hring[:, ko, sl_prev, :],
                                start=(ko == 0), stop=(ko == 3))
                    p_g = ap4.tile([128, 4, BP], F32, tag=f"pre{g}",
                                   name=f"pre{g}")
                    nc.vector.tensor_add(p_g, g_ps,
                                         cbuf[:, 4 * g:4 * (g + 1), :,
                                              tc_idx])
                    if g == 2:
                        tg = ap4.tile([128, 4, BP], F32, tag="tg")
                        nc.scalar.activation(tg, p_g, AF.Tanh)
                    elif g == 0:
                        si = ap4.tile([128, 4, BP], F32, tag="si")
                        nc.scalar.activation(si, p_g, AF.Sigmoid)
                        ig = ap4.tile([128, 4, BP], F32, tag="ig")
                        nc.vector.tensor_mul(ig, si, tg)
                    elif g == 1:
                        sf = ap4.tile([128, 4, BP], F32, tag="sf")
                        nc.scalar.activation(sf, p_g, AF.Sigmoid)
                        cf = ap4.tile([128, 4, BP], F32, tag="cf")
                        cnew = ap4.tile([128, 4, BP], F32, tag="c")
                        nc.vector.tensor_mul(cf, sf, cprev)
                        nc.vector.tensor_add(cnew, cf, ig)
                        th = ap4.tile([128, 4, BP], F32, tag="th")
                        nc.scalar.activation(th, cnew, AF.Tanh)
                    else:
                        so = ap4.tile([128, 4, BP], F32, tag="so")
                        nc.scalar.activation(so, p_g, AF.Sigmoid)
                        nc.vector.tensor_mul(hring[:, :, sl, :], so, th)
                cprev = cnew

                # filler: paced Wz ops for the upcoming chunk (3/step
                # exact), issued after the tail so tail ops don't wait
                target = 3 * (tc_idx + 1)
                while emitted < target:
                    op = next(filler, None)
                    if op is None:
                        break
                    op()
                    emitted += 1
                if tc_idx == CH - 1:
                    for op in filler:       # drain (should be empty)
                        op()
                    filler_chunk += 1
                    if filler_chunk < NCH:
                        filler = wz_chunk_ops(filler_chunk)
                    else:
                        filler = iter(())
                    emitted = 0

                if t % 16 == 15:
                    t0 = t - 15
                    sl0 = 1 + (t0 % 32)
                    s_ps = sps.tile([128, S], F32, tag="sps")
                    for ko in range(4):
                        nc.tensor.matmul(
                            s_ps, hring[:, ko, sl0:sl0 + 16, :],
                            wo_sb[:, ko, :],
                            start=(ko == 0), stop=False)
                    nc.tensor.matmul(s_ps, ones1, bout16,
                                     start=False, stop=True)
                    sig = rd.tile([128, S], F32, tag="sig")
                    nc.scalar.activation(sig, s_ps, AF.Sigmoid)
                    q_t = rd.tile([128, S], F32, tag="qt")
                    nc.sync.dma_start(
                        out=q_t,
                        in_=bass.AP(tensor=q.tensor,
                                    offset=q.offset + t0 * S,
                                    ap=[[S, 16], [T * S, BP], [1, S]]),
                    )
                    ycol = rd.tile([128, 1], F32, tag="ycol")
                    nc.vector.tensor_mul(sig, sig, q_t)
                    nc.vector.tensor_reduce(ycol, sig, mybir.AxisListType.X,
                                            ALU.add)
                    nc.sync.dma_start(out=y[t0:t0 + 16, :], in_=ycol)


_CACHE = {}


def _get_nc(T=T_FULL):
    if T not in _CACHE:
        _CACHE[T] = _build(T)
    return _CACHE[T]


def kernel(x, delta, q, W_emb, b_emb, W_lstm, U_lstm, b_lstm, W_out, b_out):
    T = x.shape[1]
    nc = _get_nc(T)
    shared = dict(
        W_emb=np.ascontiguousarray(W_emb, np.float32),
        b_emb=np.ascontiguousarray(b_emb, np.float32),
        W_lstm=np.ascontiguousarray(W_lstm, np.float32),
        U_lstm=np.ascontiguousarray(U_lstm, np.float32),
        b_lstm=np.ascontiguousarray(b_lstm, np.float32),
        W_out=np.ascontiguousarray(W_out, np.float32),
        b_out=np.ascontiguousarray(b_out, np.float32),
    )
    in_maps = []
    for c in range(N_CORES):
        sl = slice(BP * c, BP * (c + 1))
        in_maps.append(dict(
            x=np.ascontiguousarray(x[sl], np.float32),
            delta=np.ascontiguousarray(np.asarray(delta)[sl, :, 0], np.float32),
            q=np.ascontiguousarray(q[sl], np.float32),
            **shared,
        ))
    res = run_bass_kernel_spmd(nc, in_maps, core_ids=list(range(N_CORES)))
    out = np.empty((x.shape[0], T, 1), np.float32)
    for c in range(N_CORES):
        out[BP * c:BP * (c + 1), :, 0] = res.results[c]["y"].T
    return out
